# revision 10
# baseline (speedup 1.0000x reference)
"""Single-head causal attention (B=8, T=2048, D=1024, HS=64) on 8 trn2
NeuronCores, data-parallel over batch (1 batch element per core).

v3: chunk pipeline with PE-density focus.
  - host feeds x^T partition-major per 512-chunk (single descriptor per
    partition per DMA) and one packed bf16 constant block
  - all DMAs issued up front; chunk fronts (QK proj, V proj, V
    transpose) emitted as dense MM bursts to keep HAM warm
  - scores: row-packed pair of key blocks into one 2-bank psum tile;
    ONE exp op covers both halves; diagonal handled by a [zeros|tri]
    mask multiply on DVE
  - PV (one chunk behind) woven between score pairs
  - out^T accumulated with a ones-augmented V column for denominators
"""

import numpy as np
import ml_dtypes

import concourse.bass as bass
import concourse.bacc as bacc
import concourse.tile as tile
from concourse import mybir
from concourse.bass_utils import run_bass_kernel_spmd
from concourse.vector_clock import ScopedClock

B, T, D, HS = 8, 2048, 1024, 64
NCORES = 8
P = 128
ND = D // P        # 8 d-chunks
NB = T // P        # 16 t-blocks
CH = 512
NCH = T // CH      # 4 chunks

BF16 = mybir.dt.bfloat16
F32 = mybir.dt.float32

# packed bf16 constant block offsets (free-dim)
OFF_W = 0            # [128, 8, 128] wqk chunks
OFF_WV = 1024        # [128, 8, 64]  wv chunks
OFF_MASK = 1536      # [128, 256]    [zeros | tri]
OFF_IDB = 1792       # [128, 64]     bf16 identity (rows >= 64 zero)
CPACK_N = 1856

_MAX_DRAIN_WAITS = 1


def _split_drain_and_barrier(self, tick_clock, wait_clock):
    # Workaround for this walrus build rejecting >1 sem wait on the tail
    # drain: split the waits across a chain of SP nops.
    nc = self.nc
    drain_inst = nc.sync.drain()
    wait_clock.add_sem_waits(
        drain_inst.ins, ScopedClock({None: tick_clock.global_clock})
    )
    si = drain_inst.ins.sync_info
    if si is not None:
        waits = list(si.on_wait)
        if len(waits) > _MAX_DRAIN_WAITS:
            si.on_wait = waits[:_MAX_DRAIN_WAITS]
            drain_inst.ins.sync_info = si
            for i in range(_MAX_DRAIN_WAITS, len(waits), _MAX_DRAIN_WAITS):
                nop = nc.sync.nop(nofuse=True)
                nsi = nop.ins.sync_info
                if nsi is None:
                    nsi = mybir.SyncInfo(on_wait=[], on_update=[])
                nsi.on_wait = waits[i : i + _MAX_DRAIN_WAITS]
                nop.ins.sync_info = nsi

    nc.all_engine_barrier()
    assert self.sems is not None
    popped = nc._tile_sem_poison_stack.pop()
    assert popped is self._sem_poison
    nc.clear_and_free_semaphores(list(self.sems.allocated().values()))
    nc.all_engine_barrier()


tile.TileContext._drain_and_barrier = _split_drain_and_barrier


def build_kernel() -> bass.Bass:
    nc = bacc.Bacc("TRN2", target_bir_lowering=False, debug=False, num_devices=NCORES)
    # x^T partition-major per chunk: [NCH, 128, ND, CH]
    xT = nc.dram_tensor("xT", [NCH, P, ND, CH], BF16, kind="ExternalInput")
    cpack = nc.dram_tensor("cpack", [P, CPACK_N], BF16, kind="ExternalInput")
    idf = nc.dram_tensor("idf", [HS + 1, HS + 1], F32, kind="ExternalInput")
    out = nc.dram_tensor("out", [T, HS], F32, kind="ExternalOutput")

    with tile.TileContext(nc) as tc:
        with (
            tc.tile_pool(name="consts", bufs=1) as consts,
            tc.tile_pool(name="xt", bufs=1) as xpool,
            tc.tile_pool(name="qk", bufs=1) as qkpool,
            tc.tile_pool(name="vt", bufs=1) as vtpool,
            tc.tile_pool(name="v", bufs=1) as vpool,
            tc.tile_pool(name="e", bufs=22) as epool,
            tc.tile_pool(name="ot", bufs=2) as otpool,
            tc.tile_pool(name="o", bufs=2) as opool,
            tc.tile_pool(name="rcp", bufs=4) as rcppool,
            tc.tile_pool(name="proj_ps", bufs=1, space="PSUM") as ppsum,
            tc.tile_pool(name="score_ps", bufs=3, space="PSUM") as spsum,
            tc.tile_pool(name="pv_ps", bufs=1, space="PSUM") as pvpsum,
        ):
            # ---------- all DMAs up front ----------
            xt_sb = xpool.tile([P, NCH, ND, CH], BF16)
            nc.sync.dma_start(out=xt_sb[:, 0], in_=xT[0])
            cp_sb = consts.tile([P, CPACK_N], BF16)
            nc.sync.dma_start(out=cp_sb, in_=cpack[:, :])
            idf_sb = consts.tile([HS + 1, HS + 1], F32)
            nc.sync.dma_start(out=idf_sb, in_=idf[:, :])
            for ic in range(1, NCH):
                nc.sync.dma_start(out=xt_sb[:, ic], in_=xT[ic])

            w_sb = cp_sb[:, OFF_W : OFF_W + 1024].rearrange(
                "p (dc m) -> p dc m", m=P
            )
            wv_sb = cp_sb[:, OFF_WV : OFF_WV + 512].rearrange(
                "p (dc m) -> p dc m", m=HS
            )
            mask2_sb = cp_sb[:, OFF_MASK : OFF_MASK + 256]  # [zeros | tri]
            idb_sb = cp_sb[0:HS, OFF_IDB : OFF_IDB + HS]

            qkA = qkpool.tile([P, T], BF16, tag="qkA")  # Q^T top / K^T bottom
            qkB = qkpool.tile([P, T], BF16, tag="qkB")  # swapped
            vt_sb = vtpool.tile([HS, T], BF16)
            v_sb = vpool.tile([P, NB, HS + 1], BF16)
            e_tiles = {}

            def emit_front(ic):
                """QK proj + V proj + V transpose for chunk ic (dense PE)."""
                csl = slice(ic * CH, (ic + 1) * CH)
                ps = ppsum.tile([P, CH], F32, tag="proj", name=f"qkps_{ic}")
                for dc in range(ND):
                    nc.tensor.matmul(
                        ps[:],
                        w_sb[:, dc, :],
                        xt_sb[:, ic, dc, :],
                        start=(dc == 0),
                        stop=(dc == ND - 1),
                    )
                psv = ppsum.tile([HS, CH], F32, tag="proj", name=f"vps_{ic}")
                for dc in range(ND):
                    nc.tensor.matmul(
                        psv[:],
                        wv_sb[:, dc, :],
                        xt_sb[:, ic, dc, :],
                        start=(dc == 0),
                        stop=(dc == ND - 1),
                    )
                nc.vector.tensor_copy(out=qkA[:, csl], in_=ps[:])
                nc.vector.tensor_copy(out=qkB[0:HS, csl], in_=qkA[HS:P, csl])
                nc.vector.tensor_copy(out=qkB[HS:P, csl], in_=qkA[0:HS, csl])
                nc.vector.tensor_copy(out=vt_sb[:, csl], in_=psv[:])
                pst = ppsum.tile([P, 4, HS], BF16, tag="proj", name=f"vtps_{ic}")
                for q in range(4):
                    tb = 4 * ic + q
                    nc.tensor.transpose(
                        pst[:, q, :], vt_sb[:, tb * P : (tb + 1) * P], idb_sb
                    )
                nc.vector.tensor_copy(out=v_sb[:, 4 * ic : 4 * ic + 4, 0:HS], in_=pst[:])
                nc.gpsimd.memset(v_sb[:, 4 * ic : 4 * ic + 4, HS : HS + 1], 1.0)

            def emit_score_pair(ic, g):
                """Row-packed pair (jb0=2g, jb1=2g+1), one merged exp, diag
                mask on DVE."""
                jb0, jb1 = 2 * g, 2 * g + 1
                off = max(0, P * jb0 - CH * ic)
                n = CH - off
                lo, hi = CH * ic + off, (ic + 1) * CH
                psp = spsum.tile([P, 2, CH], F32, tag="score", name=f"sps_{ic}_{g}")
                nc.tensor.matmul(
                    psp[:, 0, off:CH],
                    qkB[0:HS, jb0 * P : (jb0 + 1) * P],
                    qkA[0:HS, lo:hi],
                    start=True,
                    stop=True,
                )
                nc.tensor.matmul(
                    psp[:, 1, off:CH],
                    qkA[HS:P, jb1 * P : (jb1 + 1) * P],
                    qkB[HS:P, lo:hi],
                    start=True,
                    stop=True,
                )
                et = epool.tile([P, 2, CH], BF16, tag="e", name=f"e_{ic}_{g}")
                e_tiles[(ic, g)] = et
                nc.scalar.activation(
                    out=et[:, :, off:CH],
                    in_=psp[:, :, off:CH],
                    func=mybir.ActivationFunctionType.Exp,
                    scale=float(HS) ** -0.5,
                )
                if ic == g // 2:  # diagonal pair
                    nc.vector.tensor_mul(
                        et[:, 1, off : off + 256], et[:, 1, off : off + 256], mask2_sb[:]
                    )
                    nc.vector.tensor_mul(
                        et[:, 0, off : off + P],
                        et[:, 0, off : off + P],
                        mask2_sb[:, P : 2 * P],
                    )

            def emit_pv(ic, pv_ps, jb):
                njb = 4 * ic + 4
                g = jb // 2
                off = max(0, P * (2 * g) - CH * ic)
                nc.tensor.matmul(
                    pv_ps[:, off:CH],
                    v_sb[:, jb, :],
                    e_tiles[(ic, g)][:, jb & 1, off:CH],
                    start=(jb == 0),
                    stop=(jb == njb - 1),
                )

            def emit_finalize(ic, pv_ps):
                ot = otpool.tile([HS + 1, CH], F32, tag="ot", name=f"ot_{ic}")
                nc.vector.tensor_copy(out=ot[:], in_=pv_ps[:])
                pst = ppsum.tile([P, 4, HS + 1], F32, tag="proj", name=f"fps_{ic}")
                for q in range(4):
                    nc.tensor.transpose(
                        pst[:, q, :], ot[:, q * P : (q + 1) * P], idf_sb[:]
                    )
                rcp = rcppool.tile([P, 4], F32, tag="rcp", name=f"rcp_{ic}")
                nc.vector.reciprocal(rcp[:], pst[:, :, HS])
                o_sb = opool.tile([P, 4, HS], F32, tag="o", name=f"o_{ic}")
                for q in range(4):
                    nc.vector.tensor_scalar_mul(
                        o_sb[:, q, :], pst[:, q, 0:HS], rcp[:, q : q + 1]
                    )
                nc.sync.dma_start(
                    out=out[ic * CH : (ic + 1) * CH, :].rearrange(
                        "(q p) h -> p q h", p=P
                    ),
                    in_=o_sb[:],
                )

            # ---------- pipeline ----------
            emit_front(0)
            pv_ps_of = {}
            for ic in range(NCH):
                prev = ic - 1
                pv_jbs = list(range(4 * prev + 4)) if prev >= 0 else []
                if prev >= 0:
                    pv_ps_of[prev] = pvpsum.tile(
                        [HS + 1, CH], F32, tag="pv", name=f"pvps_{prev}"
                    )
                pairs = list(range(2 * ic + 2))
                nsteps = len(pairs)
                for si_, g in enumerate(pairs):
                    emit_score_pair(ic, g)
                    lo = len(pv_jbs) * si_ // nsteps
                    hi = len(pv_jbs) * (si_ + 1) // nsteps
                    for jb in pv_jbs[lo:hi]:
                        emit_pv(prev, pv_ps_of[prev], jb)
                if prev >= 0:
                    emit_finalize(prev, pv_ps_of[prev])
                if ic + 1 < NCH:
                    emit_front(ic + 1)

            ic = NCH - 1
            pv_ps_of[ic] = pvpsum.tile([HS + 1, CH], F32, tag="pv", name=f"pvps_{ic}")
            for jb in range(4 * ic + 4):
                emit_pv(ic, pv_ps_of[ic], jb)
            emit_finalize(ic, pv_ps_of[ic])

    nc.compile()
    return nc


_NC_CACHE = None


def _get_nc():
    global _NC_CACHE
    if _NC_CACHE is None:
        _NC_CACHE = build_kernel()
    return _NC_CACHE


def _make_in_maps(inputs):
    x, Wq, Wk, Wv = inputs["x"], inputs["Wq"], inputs["Wk"], inputs["Wv"]
    assert x.shape == (B, T, D)
    bf = ml_dtypes.bfloat16

    wqk = np.concatenate([Wq, Wk], axis=1)  # [D, 128]
    cpack = np.zeros((P, CPACK_N), dtype=np.float32)
    # w: cpack[p, dc*128+m] = wqk[dc*128+p, m]
    cpack[:, OFF_W : OFF_W + 1024] = (
        wqk.reshape(ND, P, P).transpose(1, 0, 2).reshape(P, 1024)
    )
    cpack[:, OFF_WV : OFF_WV + 512] = (
        Wv.reshape(ND, P, HS).transpose(1, 0, 2).reshape(P, 512)
    )
    tri = np.triu(np.ones((P, P), dtype=np.float32))  # keep j <= i
    cpack[:, OFF_MASK : OFF_MASK + P] = 0.0
    cpack[:, OFF_MASK + P : OFF_MASK + 2 * P] = tri
    cpack[0:HS, OFF_IDB : OFF_IDB + HS] = np.eye(HS, dtype=np.float32)
    cpack = cpack.astype(bf)

    idf = np.eye(HS + 1, dtype=np.float32)

    in_maps = []
    for b in range(NCORES):
        # [NCH, P, ND, CH]: xTc[ic, p, dc, t] = x[b, ic*CH + t, dc*P + p]
        xTb = np.ascontiguousarray(
            x[b].reshape(NCH, CH, ND, P).transpose(0, 3, 2, 1)
        ).astype(bf)
        in_maps.append({"xT": xTb, "cpack": cpack, "idf": idf})
    return in_maps


def kernel(x, Wq, Wk, Wv):
    in_maps = _make_in_maps({"x": x, "Wq": Wq, "Wk": Wk, "Wv": Wv})
    nc = _get_nc()
    res = run_bass_kernel_spmd(nc, in_maps, list(range(NCORES)))
    return np.stack([res.results[b]["out"] for b in range(NCORES)], axis=0)


# revision 11
# speedup vs baseline: 1.1108x; 1.1108x over previous
"""Single-head causal attention (B=8, T=2048, D=1024, HS=64) on 8 trn2
NeuronCores, data-parallel over batch (1 batch element per core).

v3: chunk pipeline with PE-density focus.
  - host feeds x^T partition-major per 512-chunk (single descriptor per
    partition per DMA) and one packed bf16 constant block
  - all DMAs issued up front; chunk fronts (QK proj, V proj, V
    transpose) emitted as dense MM bursts to keep HAM warm
  - scores: row-packed pair of key blocks into one 2-bank psum tile;
    ONE exp op covers both halves; diagonal handled by a [zeros|tri]
    mask multiply on DVE
  - PV (one chunk behind) woven between score pairs
  - out^T accumulated with a ones-augmented V column for denominators
"""

import numpy as np
import ml_dtypes

import concourse.bass as bass
import concourse.bacc as bacc
import concourse.tile as tile
from concourse import mybir
from concourse.bass_utils import run_bass_kernel_spmd
from concourse.vector_clock import ScopedClock

B, T, D, HS = 8, 2048, 1024, 64
NCORES = 8
P = 128
ND = D // P        # 8 d-chunks
NB = T // P        # 16 t-blocks
CH = 512
NCH = T // CH      # 4 chunks

BF16 = mybir.dt.bfloat16
F32 = mybir.dt.float32

# packed bf16 constant block offsets (free-dim)
OFF_W = 0            # [128, 8, 128] wqk chunks
OFF_WV = 1024        # [128, 8, 64]  wv chunks
OFF_MASK = 1536      # [128, 256]    [zeros | tri]
OFF_IDB = 1792       # [128, 64]     bf16 identity (rows >= 64 zero)
CPACK_N = 1856

_MAX_DRAIN_WAITS = 1


def _split_drain_and_barrier(self, tick_clock, wait_clock):
    # Workaround for this walrus build rejecting >1 sem wait on the tail
    # drain: split the waits across a chain of SP nops.
    nc = self.nc
    drain_inst = nc.sync.drain()
    wait_clock.add_sem_waits(
        drain_inst.ins, ScopedClock({None: tick_clock.global_clock})
    )
    si = drain_inst.ins.sync_info
    if si is not None:
        waits = list(si.on_wait)
        if len(waits) > _MAX_DRAIN_WAITS:
            si.on_wait = waits[:_MAX_DRAIN_WAITS]
            drain_inst.ins.sync_info = si
            for i in range(_MAX_DRAIN_WAITS, len(waits), _MAX_DRAIN_WAITS):
                nop = nc.sync.nop(nofuse=True)
                nsi = nop.ins.sync_info
                if nsi is None:
                    nsi = mybir.SyncInfo(on_wait=[], on_update=[])
                nsi.on_wait = waits[i : i + _MAX_DRAIN_WAITS]
                nop.ins.sync_info = nsi

    nc.all_engine_barrier()
    assert self.sems is not None
    popped = nc._tile_sem_poison_stack.pop()
    assert popped is self._sem_poison
    nc.clear_and_free_semaphores(list(self.sems.allocated().values()))
    nc.all_engine_barrier()


tile.TileContext._drain_and_barrier = _split_drain_and_barrier


def build_kernel() -> bass.Bass:
    nc = bacc.Bacc("TRN2", target_bir_lowering=False, debug=False, num_devices=NCORES)
    # x^T partition-major per chunk: [NCH, 128, ND, CH]
    xT = nc.dram_tensor("xT", [NCH, P, ND, CH], BF16, kind="ExternalInput")
    cpack = nc.dram_tensor("cpack", [P, CPACK_N], BF16, kind="ExternalInput")
    idf = nc.dram_tensor("idf", [HS + 1, HS + 1], F32, kind="ExternalInput")
    out = nc.dram_tensor("out", [T, HS], F32, kind="ExternalOutput")

    with tile.TileContext(nc) as tc:
        with (
            tc.tile_pool(name="consts", bufs=1) as consts,
            tc.tile_pool(name="xt", bufs=1) as xpool,
            tc.tile_pool(name="qk", bufs=1) as qkpool,
            tc.tile_pool(name="vt", bufs=1) as vtpool,
            tc.tile_pool(name="v", bufs=1) as vpool,
            tc.tile_pool(name="e", bufs=22) as epool,
            tc.tile_pool(name="ot", bufs=2) as otpool,
            tc.tile_pool(name="o", bufs=2) as opool,
            tc.tile_pool(name="rcp", bufs=4) as rcppool,
            tc.tile_pool(name="proj_ps", bufs=2, space="PSUM") as ppsum,
            tc.tile_pool(name="score_ps", bufs=2, space="PSUM") as spsum,
            tc.tile_pool(name="pv_ps", bufs=2, space="PSUM") as pvpsum,
        ):
            # ---------- all DMAs up front ----------
            xt_sb = xpool.tile([P, NCH, ND, CH], BF16)
            nc.sync.dma_start(out=xt_sb[:, 0], in_=xT[0])
            cp_sb = consts.tile([P, CPACK_N], BF16)
            nc.sync.dma_start(out=cp_sb, in_=cpack[:, :])
            idf_sb = consts.tile([HS + 1, HS + 1], F32)
            nc.sync.dma_start(out=idf_sb, in_=idf[:, :])
            for ic in range(1, NCH):
                nc.sync.dma_start(out=xt_sb[:, ic], in_=xT[ic])

            w_sb = cp_sb[:, OFF_W : OFF_W + 1024].rearrange(
                "p (dc m) -> p dc m", m=P
            )
            wv_sb = cp_sb[:, OFF_WV : OFF_WV + 512].rearrange(
                "p (dc m) -> p dc m", m=HS
            )
            mask2_sb = cp_sb[:, OFF_MASK : OFF_MASK + 256]  # [zeros | tri]
            idb_sb = cp_sb[0:HS, OFF_IDB : OFF_IDB + HS]

            qkA = qkpool.tile([P, T], BF16, tag="qkA")  # Q^T top / K^T bottom
            qkB = qkpool.tile([P, T], BF16, tag="qkB")  # swapped
            vt_sb = vtpool.tile([HS, T], BF16)
            v_sb = vpool.tile([P, NB, HS + 1], BF16)
            e_tiles = {}

            def emit_front(ic):
                """QK proj + V proj + V transpose for chunk ic (dense PE)."""
                csl = slice(ic * CH, (ic + 1) * CH)
                ps = ppsum.tile([P, CH], F32, tag="proj", name=f"qkps_{ic}")
                for dc in range(ND):
                    nc.tensor.matmul(
                        ps[:],
                        w_sb[:, dc, :],
                        xt_sb[:, ic, dc, :],
                        start=(dc == 0),
                        stop=(dc == ND - 1),
                    )
                psv = ppsum.tile([HS, CH], F32, tag="proj", name=f"vps_{ic}")
                for dc in range(ND):
                    nc.tensor.matmul(
                        psv[:],
                        wv_sb[:, dc, :],
                        xt_sb[:, ic, dc, :],
                        start=(dc == 0),
                        stop=(dc == ND - 1),
                    )
                nc.vector.tensor_copy(out=qkA[:, csl], in_=ps[:])
                nc.vector.tensor_copy(out=qkB[0:HS, csl], in_=qkA[HS:P, csl])
                nc.vector.tensor_copy(out=qkB[HS:P, csl], in_=qkA[0:HS, csl])
                nc.vector.tensor_copy(out=vt_sb[:, csl], in_=psv[:])
                pst = ppsum.tile([P, 4, HS], BF16, tag="proj", name=f"vtps_{ic}")
                for q in range(4):
                    tb = 4 * ic + q
                    nc.tensor.transpose(
                        pst[:, q, :], vt_sb[:, tb * P : (tb + 1) * P], idb_sb
                    )
                nc.vector.tensor_copy(out=v_sb[:, 4 * ic : 4 * ic + 4, 0:HS], in_=pst[:])
                nc.gpsimd.memset(v_sb[:, 4 * ic : 4 * ic + 4, HS : HS + 1], 1.0)

            def emit_score_pair(ic, g):
                """Row-packed pair (jb0=2g, jb1=2g+1), one merged exp, diag
                mask on DVE."""
                jb0, jb1 = 2 * g, 2 * g + 1
                off = max(0, P * jb0 - CH * ic)
                n = CH - off
                lo, hi = CH * ic + off, (ic + 1) * CH
                psp = spsum.tile([P, 2, CH], F32, tag="score", name=f"sps_{ic}_{g}")
                nc.tensor.matmul(
                    psp[:, 0, off:CH],
                    qkB[0:HS, jb0 * P : (jb0 + 1) * P],
                    qkA[0:HS, lo:hi],
                    start=True,
                    stop=True,
                )
                nc.tensor.matmul(
                    psp[:, 1, off:CH],
                    qkA[HS:P, jb1 * P : (jb1 + 1) * P],
                    qkB[HS:P, lo:hi],
                    start=True,
                    stop=True,
                )
                et = epool.tile([P, 2, CH], BF16, tag="e", name=f"e_{ic}_{g}")
                e_tiles[(ic, g)] = et
                nc.scalar.activation(
                    out=et[:, :, off:CH],
                    in_=psp[:, :, off:CH],
                    func=mybir.ActivationFunctionType.Exp,
                    scale=float(HS) ** -0.5,
                )
                if ic == g // 2:  # diagonal pair
                    nc.vector.tensor_mul(
                        et[:, 1, off : off + 256], et[:, 1, off : off + 256], mask2_sb[:]
                    )
                    nc.vector.tensor_mul(
                        et[:, 0, off : off + P],
                        et[:, 0, off : off + P],
                        mask2_sb[:, P : 2 * P],
                    )

            def emit_pv(ic, pv_ps, jb):
                njb = 4 * ic + 4
                g = jb // 2
                off = max(0, P * (2 * g) - CH * ic)
                nc.tensor.matmul(
                    pv_ps[:, off:CH],
                    v_sb[:, jb, :],
                    e_tiles[(ic, g)][:, jb & 1, off:CH],
                    start=(jb == 0),
                    stop=(jb == njb - 1),
                )

            def emit_finalize(ic, pv_ps):
                ot = otpool.tile([HS + 1, CH], F32, tag="ot", name=f"ot_{ic}")
                nc.vector.tensor_copy(out=ot[:], in_=pv_ps[:])
                pst = ppsum.tile([P, 4, HS + 1], F32, tag="proj", name=f"fps_{ic}")
                for q in range(4):
                    nc.tensor.transpose(
                        pst[:, q, :], ot[:, q * P : (q + 1) * P], idf_sb[:]
                    )
                rcp = rcppool.tile([P, 4], F32, tag="rcp", name=f"rcp_{ic}")
                nc.vector.reciprocal(rcp[:], pst[:, :, HS])
                o_sb = opool.tile([P, 4, HS], F32, tag="o", name=f"o_{ic}")
                for q in range(4):
                    nc.vector.tensor_scalar_mul(
                        o_sb[:, q, :], pst[:, q, 0:HS], rcp[:, q : q + 1]
                    )
                nc.sync.dma_start(
                    out=out[ic * CH : (ic + 1) * CH, :].rearrange(
                        "(q p) h -> p q h", p=P
                    ),
                    in_=o_sb[:],
                )

            # ---------- pipeline ----------
            emit_front(0)
            pv_ps_of = {}
            for ic in range(NCH):
                prev = ic - 1
                pv_jbs = list(range(4 * prev + 4)) if prev >= 0 else []
                if prev >= 0:
                    pv_ps_of[prev] = pvpsum.tile(
                        [HS + 1, CH], F32, tag="pv", name=f"pvps_{prev}"
                    )
                pairs = list(range(2 * ic + 2))
                nsteps = len(pairs)
                for si_, g in enumerate(pairs):
                    emit_score_pair(ic, g)
                    lo = len(pv_jbs) * si_ // nsteps
                    hi = len(pv_jbs) * (si_ + 1) // nsteps
                    for jb in pv_jbs[lo:hi]:
                        emit_pv(prev, pv_ps_of[prev], jb)
                if prev >= 0:
                    emit_finalize(prev, pv_ps_of[prev])
                if ic + 1 < NCH:
                    emit_front(ic + 1)

            ic = NCH - 1
            pv_ps_of[ic] = pvpsum.tile([HS + 1, CH], F32, tag="pv", name=f"pvps_{ic}")
            for jb in range(4 * ic + 4):
                emit_pv(ic, pv_ps_of[ic], jb)
            emit_finalize(ic, pv_ps_of[ic])

    nc.compile()
    return nc


_NC_CACHE = None


def _get_nc():
    global _NC_CACHE
    if _NC_CACHE is None:
        _NC_CACHE = build_kernel()
    return _NC_CACHE


def _make_in_maps(inputs):
    x, Wq, Wk, Wv = inputs["x"], inputs["Wq"], inputs["Wk"], inputs["Wv"]
    assert x.shape == (B, T, D)
    bf = ml_dtypes.bfloat16

    wqk = np.concatenate([Wq, Wk], axis=1)  # [D, 128]
    cpack = np.zeros((P, CPACK_N), dtype=np.float32)
    # w: cpack[p, dc*128+m] = wqk[dc*128+p, m]
    cpack[:, OFF_W : OFF_W + 1024] = (
        wqk.reshape(ND, P, P).transpose(1, 0, 2).reshape(P, 1024)
    )
    cpack[:, OFF_WV : OFF_WV + 512] = (
        Wv.reshape(ND, P, HS).transpose(1, 0, 2).reshape(P, 512)
    )
    tri = np.triu(np.ones((P, P), dtype=np.float32))  # keep j <= i
    cpack[:, OFF_MASK : OFF_MASK + P] = 0.0
    cpack[:, OFF_MASK + P : OFF_MASK + 2 * P] = tri
    cpack[0:HS, OFF_IDB : OFF_IDB + HS] = np.eye(HS, dtype=np.float32)
    cpack = cpack.astype(bf)

    idf = np.eye(HS + 1, dtype=np.float32)

    in_maps = []
    for b in range(NCORES):
        # [NCH, P, ND, CH]: xTc[ic, p, dc, t] = x[b, ic*CH + t, dc*P + p]
        xTb = np.ascontiguousarray(
            x[b].reshape(NCH, CH, ND, P).transpose(0, 3, 2, 1)
        ).astype(bf)
        in_maps.append({"xT": xTb, "cpack": cpack, "idf": idf})
    return in_maps


def kernel(x, Wq, Wk, Wv):
    in_maps = _make_in_maps({"x": x, "Wq": Wq, "Wk": Wk, "Wv": Wv})
    nc = _get_nc()
    res = run_bass_kernel_spmd(nc, in_maps, list(range(NCORES)))
    return np.stack([res.results[b]["out"] for b in range(NCORES)], axis=0)
